# revision 57
# baseline (speedup 1.0000x reference)
"""Trainium2 Bass kernel for a 2-layer sparse GAT (nn_GAT_71889162600962).

Strategy (8 NeuronCores, SPMD):
- Nodes striped across cores (12500/core). Edges sharded by the core that
  owns their *src* node, so each core exclusively owns the segment sums
  (num/denom) of its stripe -- no all-reduce.
- Per layer, each core computes its stripe of h2 = h @ W (and the two
  attention projections s_src/s_dst = h2 . a halves) with bf16 PE matmuls,
  transposes h2 back to row-major "records"
      rec[n] = [h2[n] (256 bf16) | s_dst[n] | 1.0 | pad] (768B rows)
  plus a [stripe,1] f32 s_src side table, and all-gathers the record
  table across cores (vfull = 8*stripe rows).
- Edge phase: edges sorted by src, packed into chunks of <=128
  consecutive src nodes x 4096 edge slots, the slots split into 4
  1024-slot cells by dst quartile (dma_gather int16 indices; 1024 is the
  per-instruction descriptor-carveout limit). Per chunk: 4 dma_gather
  instructions fetch rec[dst] for all slots (slot i -> partition i%128,
  tile i//128); one indirect DMA with equal per-partition offsets
  broadcasts the chunk's 128-row s_src window to all partitions. Scores
  for all 32 tiles are then built as full [128e x 128s] matrices in 4
  fused DVE passes + 1 ACT exp:
      m[e,t,s] = (iota[s]==src_local[e,t]) * exp(-lrelu(ssrc[s]+sdst[e,t]))
  (stride-0 broadcast access patterns), and 32 PE matmuls accumulate
  num/denom in PSUM via psum[s,:] += m[:,t,:]^T @ X[:,t,:258]. Finalize
  (num/denom, ELU) per chunk; rows land via a 128-row indirect scatter.
  Pad slots point at the stripe-1 sentinel record whose s_dst is BIG, so
  exp(-lrelu(BIG+x)) == 0 kills their contribution.
- All per-core variation lives in input index arrays, so one SPMD program
  serves all 8 cores. Inputs are device_put once; steady-state reruns
  measure device execution (amortized over async dispatches).
"""

import math

import numpy as np
import ml_dtypes

P = 128
D = 256
REC_W = 384              # record row (bf16): 256 h2 | s_dst | 1.0 | s_src | pad
NCORES = 8
NEG_SLOPE = 0.2
BIG = 200.0              # sentinel score -> exp(-BIG) == 0 in f32
CELL = 1024              # slots per dst-quartile cell (dma_gather int16 idx)
NCELLS = 4
CHUNK_SLOTS = CELL * NCELLS   # 4096 edge slots per chunk (32 tiles of 128)
TILES_PER_CHUNK = CHUNK_SLOTS // P
TILES_PER_CELL = CELL // P

_LAST_RESULTS = None
TRACE = False

_IOTA_F = np.tile(np.arange(P, dtype=np.float32)[None, :], (P, 1))
_IDENT_BF = np.eye(P, dtype=np.float32).astype(ml_dtypes.bfloat16)


def _cfg(n_nodes):
    npc = n_nodes // NCORES
    stripe = math.ceil((npc + 44) / 128) * 128
    return npc, stripe, stripe * NCORES


# ---------------------------------------------------------------------------
# Host-side preprocessing
# ---------------------------------------------------------------------------

def _pack_core(src_l, dst_g, npc, stripe, qrows):
    """Pack one core's (src-local, dst-global-row) edges, sorted by src,
    into chunks of <=128 consecutive nodes whose per-dst-quartile edge
    counts each fit a CELL."""
    order = np.argsort(src_l, kind="stable")
    s = src_l[order]
    d = dst_g[order]
    q = d // qrows
    ne = len(s)

    starts = np.flatnonzero(np.r_[True, s[1:] != s[:-1]])
    seg_node = s[starts]
    seg_len = np.diff(np.r_[starts, ne])
    nseg = len(starts)
    seg_id = np.repeat(np.arange(nseg), seg_len)
    cnt = np.bincount(seg_id * NCELLS + q,
                      minlength=nseg * NCELLS).reshape(nseg, NCELLS)

    chunks = []  # (node_base, n_nodes, seg_start, seg_end)
    i = 0
    while i < nseg:
        base = int(seg_node[i])
        i0 = i
        cell = np.zeros(NCELLS, np.int64)
        while i < nseg and int(seg_node[i]) - base < P:
            nxt = cell + cnt[i]
            if (nxt > CELL).any():
                break
            cell = nxt
            i += 1
        chunks.append((base, int(seg_node[i - 1]) - base + 1, i0, i))
    return {"chunks": chunks, "s": s, "d": d, "q": q, "starts": starts,
            "seg_len": seg_len}


def _fill_arrays(packed, n_chunks, stripe, qrows):
    s, dd, qq = packed["s"], packed["d"], packed["q"]
    starts, seg_len = packed["starts"], packed["seg_len"]
    chunks = packed["chunks"]
    C = n_chunks
    sent = stripe - 1
    TPC = TILES_PER_CHUNK

    xidx = np.full((C, NCELLS, CELL), sent, np.int16)  # rec row rel. quartile
    scol = np.full((C, CHUNK_SLOTS), -1.0, np.float32)  # src - off in [0,128)
    oidx = np.full((C, P), sent, np.int32)
    obase = np.zeros((C, P), np.int32)                 # ssrc window base

    for k, (base, nn, i0, i1) in enumerate(chunks):
        e0 = int(starts[i0])
        e1 = int(starts[i1 - 1] + seg_len[i1 - 1])
        dce, sce, qce = dd[e0:e1], s[e0:e1], qq[e0:e1]
        off = min(base, stripe - P)
        for q in range(NCELLS):
            sel = qce == q
            n = int(sel.sum())
            xidx[k, q, :n] = (dce[sel] - q * qrows).astype(np.int16)
            scol[k, q * CELL:q * CELL + n] = (sce[sel] - off)
        lo = base - off
        oidx[k, lo:lo + nn] = base + np.arange(nn, dtype=np.int32)
        obase[k, :] = off

    # wrap16: index flat position i -> [i % 16, i // 16], replicated x8
    def w16(a):
        n = a.shape[-1]
        out = a.reshape(-1, n // 16, 16).transpose(0, 2, 1)
        return np.ascontiguousarray(
            np.tile(out, (1, 8, 1))).reshape(*a.shape[:-1], P, n // 16)

    # slot i -> (partition i % 128, tile i // 128): [C, 128, 32]
    def pb(a):
        return np.ascontiguousarray(
            a.reshape(C, TPC, P).transpose(0, 2, 1))

    return {"xidx": np.ascontiguousarray(w16(xidx).transpose(0, 2, 1, 3)),
            "scol": pb(scol),
            "oidx": np.ascontiguousarray(oidx[:, :, None]),
            "obase": np.ascontiguousarray(obase[:, :, None])}


def _prep(edges, n_nodes):
    npc, stripe, vfull = _cfg(n_nodes)
    qrows = vfull // NCELLS
    src = np.asarray(edges[0])
    dst = np.asarray(edges[1])
    core = src // npc
    dst_g = (dst // npc) * stripe + dst % npc

    packed = []
    for c in range(NCORES):
        sel = core == c
        packed.append(_pack_core(src[sel] - c * npc, dst_g[sel],
                                 npc, stripe, qrows))
    c_max = max(len(p["chunks"]) for p in packed)
    return [_fill_arrays(p, c_max, stripe, qrows) for p in packed], c_max


# ---------------------------------------------------------------------------
# Device program
# ---------------------------------------------------------------------------

def _build_program(n_nodes, n_chunks, mode=4, c_run=None):
    import concourse.bacc as bacc
    import concourse.mybir as mybir
    import concourse.tile as tile

    f32 = mybir.dt.float32
    bf16 = mybir.dt.bfloat16
    i16 = mybir.dt.int16
    Alu = mybir.AluOpType
    Act = mybir.ActivationFunctionType

    npc, stripe, vfull = _cfg(n_nodes)
    C = n_chunks
    NT = 512
    TPC = TILES_PER_CHUNK
    groups = [list(range(NCORES))]
    Off = __import__("concourse.bass", fromlist=["IndirectOffsetOnAxis"]).IndirectOffsetOnAxis

    nc = bacc.Bacc("TRN2", target_bir_lowering=False, debug=False,
                   num_devices=NCORES)

    embT_d = nc.dram_tensor("embT", [D, stripe], bf16, kind="ExternalInput")
    iota_d = nc.dram_tensor("iotaf", [P, P], f32, kind="ExternalInput")
    ident_d = nc.dram_tensor("identbf", [P, P], bf16, kind="ExternalInput")
    W_d = [nc.dram_tensor(f"W{L + 1}", [D, D], bf16, kind="ExternalInput")
           for L in range(2)]
    Wa_d = [nc.dram_tensor(f"Wa{L + 1}", [D, 2], bf16, kind="ExternalInput")
            for L in range(2)]
    i32 = mybir.dt.int32
    qrows = vfull // NCELLS
    xidx_d = nc.dram_tensor("xidx", [C, P, NCELLS, CELL // 16], i16,
                            kind="ExternalInput")
    oidx_d = nc.dram_tensor("oidx", [C, P, 1], i32, kind="ExternalInput")
    obase_d = nc.dram_tensor("obase", [C, P, 1], i32, kind="ExternalInput")
    scol_d = nc.dram_tensor("scol", [C, P, TPC], f32, kind="ExternalInput")
    out_d = nc.dram_tensor("out_stripe", [stripe, D], f32,
                           kind="ExternalOutput")

    rec_stripe = [nc.dram_tensor(f"rec_stripe{L}", [stripe, REC_W], bf16)
                  for L in range(2)]
    rec_full = [nc.dram_tensor(f"rec_full{L}", [vfull, REC_W], bf16,
                               addr_space="Shared") for L in range(2)]
    ssrc_loc = [nc.dram_tensor(f"ssrc_loc{L}", [stripe, 1], f32)
                for L in range(2)]
    out1rec = nc.dram_tensor("out1rec", [stripe, D], bf16)

    BAP = __import__("concourse.bass", fromlist=["AP"]).AP

    with tile.TileContext(nc) as tc:
        r_cell = nc.gpsimd.to_reg(CELL)
        with tc.tile_pool(name="const", bufs=1) as cpool:
            iota_f = cpool.tile([P, P], f32)
            nc.sync.dma_start(iota_f[:], iota_d[:])
            ident = cpool.tile([P, P], bf16)
            nc.sync.dma_start(ident[:], ident_d[:])
            W_sb, Wa_sb = [], []
            for L in range(2):
                w = cpool.tile([P, 2, D], bf16)
                wa = cpool.tile([P, 2, 2], bf16)
                for kc in range(2):
                    nc.sync.dma_start(w[:, kc, :], W_d[L][P * kc:P * (kc + 1)])
                    nc.sync.dma_start(wa[:, kc, :],
                                      Wa_d[L][P * kc:P * (kc + 1)])
                W_sb.append(w)
                Wa_sb.append(wa)

            for L in range(2 if mode >= 1 else 0):
                # ---------------- phase A: stripe matmul ------------------
                with (
                    tc.tile_pool(name=f"A{L}", bufs=3) as ap,
                    tc.tile_pool(name=f"As{L}", bufs=1) as spl,
                    tc.tile_pool(name=f"Ap{L}", bufs=2, space="PSUM") as pp,
                    tc.tile_pool(name=f"ApT{L}", bufs=4, space="PSUM") as ppT,
                ):
                    s_sbuf = spl.tile([3, stripe], f32)
                    nc.vector.memset(s_sbuf[:], 1.0)
                    s_bf = spl.tile([2, stripe], bf16)
                    ones_bf = spl.tile([1, stripe], bf16)
                    nc.vector.memset(ones_bf[:], 1.0)
                    for c0 in range(0, stripe, NT):
                        nsz = min(NT, stripe - c0)
                        hT = []
                        for kc in range(2):
                            t = ap.tile([P, nsz], bf16, tag="hT")
                            if L == 0:
                                nc.sync.dma_start(
                                    t[:], embT_d[P * kc:P * (kc + 1),
                                                 c0:c0 + nsz])
                            else:
                                nc.sync.dma_start_transpose(
                                    t[:], out1rec[c0:c0 + nsz,
                                                  P * kc:P * (kc + 1)])
                            hT.append(t)
                        ps_s = pp.tile([2, NT], f32, space="PSUM", tag="ps_s")
                        for kc in range(2):
                            nc.tensor.matmul(ps_s[:, :nsz],
                                             lhsT=Wa_sb[L][:, kc, :],
                                             rhs=hT[kc][:], start=kc == 0,
                                             stop=kc == 1)
                        nc.vector.tensor_copy(s_sbuf[0:2, c0:c0 + nsz],
                                              ps_s[:, :nsz])
                        nc.vector.tensor_copy(s_bf[0:2, c0:c0 + nsz],
                                              ps_s[0:2, :nsz])
                        rows = [ap.tile([P, D], bf16, tag=f"rows{b}",
                                        name=f"rows{b}")
                                for b in range(nsz // P)]
                        for j in range(2):
                            ps_h = pp.tile([P, NT], f32, space="PSUM",
                                           tag="ps_h")
                            for kc in range(2):
                                nc.tensor.matmul(
                                    ps_h[:, :nsz],
                                    lhsT=W_sb[L][:, kc, P * j:P * (j + 1)],
                                    rhs=hT[kc][:], start=kc == 0, stop=kc == 1)
                            h2T = ap.tile([P, nsz], bf16, tag="h2T")
                            nc.vector.tensor_copy(h2T[:], ps_h[:, :nsz])
                            for b in range(nsz // P):
                                psT = ppT.tile([P, P], bf16, space="PSUM",
                                               tag="psT")
                                nc.tensor.transpose(
                                    out=psT[:], in_=h2T[:, P * b:P * (b + 1)],
                                    identity=ident[:])
                                nc.vector.tensor_copy(
                                    rows[b][:, P * j:P * (j + 1)], psT[:])
                        for b in range(nsz // P):
                            nc.scalar.dma_start(
                                rec_stripe[L][c0 + P * b:c0 + P * (b + 1),
                                              :D],
                                rows[b][:])
                    nc.vector.memset(s_sbuf[0:2, stripe - 1:stripe], BIG)
                    nc.vector.memset(s_bf[0:2, stripe - 1:stripe], BIG)
                    # s_src -> local table; s_dst, 1.0 -> record cols 256/257
                    # (bf16-staged so the DMAs stay on HWDGE engines)
                    nc.sync.dma_start(ssrc_loc[L][:, 0:1], s_sbuf[0:1, :])
                    nc.scalar.dma_start(rec_stripe[L][:, D:D + 1], s_bf[1:2, :])
                    nc.scalar.dma_start(rec_stripe[L][:, D + 1:D + 2],
                                        ones_bf[0:1, :])
                    if mode >= 2:
                        nc.gpsimd.collective_compute(
                            "AllGather", Alu.bypass, replica_groups=groups,
                            ins=[rec_stripe[L][:]], outs=[rec_full[L][:]])

                # ---------------- phase B: edge phase ---------------------
                if mode < 3:
                    continue
                tgt = out1rec if L == 0 else out_d
                stage_dt = bf16 if L == 0 else f32
                with (
                    tc.tile_pool(name=f"B{L}", bufs=3) as ep,
                    tc.tile_pool(name=f"Bx{L}", bufs=3) as xp,
                    tc.tile_pool(name=f"Bm{L}", bufs=2) as mp,
                    tc.tile_pool(name=f"Bo{L}", bufs=3) as ohp,
                    tc.tile_pool(name=f"Bf{L}", bufs=2) as fp,
                    tc.tile_pool(name=f"Bp{L}", bufs=2, space="PSUM") as pnp,
                ):
                    def front(ch):
                        """Loads + Pool gathers for one chunk (issued 2
                        chunks ahead so the scatter of chunk k never stalls
                        the Pool queue behind k's compute chain)."""
                        xi = ep.tile([P, NCELLS, CELL // 16], i16, tag="xi")
                        nc.sync.dma_start(xi[:], xidx_d[ch])
                        scl = ep.tile([P, TPC], f32, tag="scl")
                        nc.sync.dma_start(scl[:], scol_d[ch])
                        oi = ep.tile([P, 1], i32, tag="oi")
                        nc.sync.dma_start(oi[:], oidx_d[ch])
                        obs = ep.tile([P, 1], i32, tag="obs")
                        nc.sync.dma_start(obs[:], obase_d[ch])

                        # one-hot built up front: depends only on scl, so
                        # DVE does it while the gathers are in flight
                        scl_ap = scl[:, :]
                        scl_b = BAP(scl_ap.tensor, scl_ap.offset,
                                    [scl_ap.ap[0], scl_ap.ap[1], [0, P]])
                        io_ap = iota_f[:, :]
                        io_b = BAP(io_ap.tensor, io_ap.offset,
                                   [io_ap.ap[0], [0, TPC], io_ap.ap[1]])
                        m_all = ohp.tile([P, TPC, P], bf16, tag="m_all")
                        nc.vector.tensor_tensor(
                            out=m_all[:], in0=io_b, in1=scl_b,
                            op=Alu.is_equal)

                        # 768B record gather by dst (4 quartile cells) and
                        # the chunk's 128-row s_src window broadcast to all
                        # partitions (all indices equal -> 512B rows)
                        X = xp.tile([P, TPC, REC_W], bf16, tag="X")
                        ssb = ep.tile([P, P], f32, tag="ssb")
                        if mode >= 4:
                            TQ = TILES_PER_CELL
                            for cq in range(NCELLS):
                                nc.gpsimd.dma_gather(
                                    out_ap=X[:, TQ * cq:TQ * (cq + 1), :],
                                    in_ap=rec_full[L][qrows * cq:
                                                      qrows * (cq + 1), :],
                                    idxs_ap=xi[:, cq, :], num_idxs=CELL,
                                    num_idxs_reg=r_cell, elem_size=REC_W)
                            nc.gpsimd.indirect_dma_start(
                                out=ssb[:], out_offset=None,
                                in_=ssrc_loc[L][:],
                                in_offset=Off(ap=obs[:, 0:1], axis=0))
                        else:
                            nc.vector.memset(X[:], 0.0)
                            nc.vector.memset(ssb[:], BIG)
                        return X, ssb, m_all, oi

                    def back(X, ssb, m_all, oi):
                        # score matrices for all 32 tiles in 4 fused ops:
                        # sm[e,t,s] = lrelu(ssrc[off+s] + s_dst[dst_e])
                        # m[e,t,s] = (iota[s]==scl[e,t]) * exp(-sm)
                        sd_ap = X[:, :, D:D + 1]
                        sd_b = BAP(sd_ap.tensor, sd_ap.offset,
                                   [sd_ap.ap[0], sd_ap.ap[1], [0, P]])
                        sb_ap = ssb[:, :]
                        sb_b = BAP(sb_ap.tensor, sb_ap.offset,
                                   [sb_ap.ap[0], [0, TPC], sb_ap.ap[1]])
                        sm = mp.tile([P, TPC, P], bf16, tag="sm")
                        nc.vector.tensor_tensor(
                            out=sm[:], in0=sb_b, in1=sd_b, op=Alu.add)
                        nc.vector.scalar_tensor_tensor(
                            out=sm[:], in0=sm[:], scalar=NEG_SLOPE,
                            in1=sm[:], op0=Alu.mult, op1=Alu.max)
                        e2 = mp.tile([P, TPC, P], bf16, tag="e2")
                        nc.scalar.activation(e2[:], sm[:], Act.Exp,
                                             scale=-1.0)
                        nc.vector.tensor_tensor(
                            out=m_all[:], in0=m_all[:], in1=e2[:],
                            op=Alu.mult)

                        psum = pnp.tile([P, D + 2], f32, space="PSUM",
                                        tag="psum")
                        for t in range(TPC):
                            nc.tensor.matmul(
                                psum[:], lhsT=m_all[:, t, :],
                                rhs=X[:, t, :D + 2],
                                start=t == 0, stop=t == TPC - 1)
                        recip = fp.tile([P, 1], f32, tag="recip")
                        nc.vector.reciprocal(recip[:], psum[:, D + 1:D + 2])
                        q = fp.tile([P, D], f32, tag="q")
                        nc.vector.tensor_scalar_mul(q[:], psum[:, :D],
                                                    recip[:, :1])
                        amin = fp.tile([P, D], f32, tag="amin")
                        nc.vector.tensor_scalar(
                            out=amin[:], in0=q[:], scalar1=0.0,
                            scalar2=None, op0=Alu.min)
                        ea = fp.tile([P, D], f32, tag="ea")
                        nc.scalar.activation(ea[:], amin[:], Act.Exp)
                        bmax = fp.tile([P, D], f32, tag="bmax")
                        nc.vector.tensor_scalar(
                            out=bmax[:], in0=q[:], scalar1=0.0,
                            scalar2=-1.0, op0=Alu.max, op1=Alu.add)
                        stage = fp.tile([P, D], stage_dt, tag="stage")
                        nc.vector.tensor_tensor(
                            out=stage[:], in0=ea[:], in1=bmax[:],
                            op=Alu.add)
                        nc.gpsimd.indirect_dma_start(
                            out=tgt[:],
                            out_offset=Off(ap=oi[:, 0:1], axis=0),
                            in_=stage[:], in_offset=None)

                    CR = C if c_run is None else min(C, c_run)
                    pend = []
                    for ch in range(CR):
                        pend.append(front(ch))
                        if len(pend) > 2:
                            back(*pend.pop(0))
                    for item in pend:
                        back(*item)
    nc.compile()
    return nc



# ---------------------------------------------------------------------------
# Persistent-jit PJRT runner (NTFF profiling is unavailable under this axon
# setup, so steady-state re-execution wall clock is the timing source).
# ---------------------------------------------------------------------------

class _Runner:
    def __init__(self, nc, n_cores):
        import jax
        from jax.sharding import Mesh, PartitionSpec
        from jax.experimental.shard_map import shard_map
        import concourse.mybir as mybir
        from concourse import bass2jax

        bass2jax.install_neuronx_cc_hook()
        self.n_cores = n_cores
        in_names, out_names, out_avals, zero_outs = [], [], [], []
        for alloc in nc.m.functions[0].allocations:
            if not isinstance(alloc, mybir.MemoryLocationSet):
                continue
            name = alloc.memorylocations[0].name
            if alloc.kind == "ExternalInput":
                in_names.append(name)
            elif alloc.kind == "ExternalOutput":
                out_names.append(name)
                shape = tuple(alloc.tensor_shape)
                dtype = mybir.dt.np(alloc.dtype)
                out_avals.append(jax.core.ShapedArray(shape, dtype))
                zero_outs.append(np.zeros(shape, dtype))
        self.partition_name = (nc.partition_id_tensor.name
                               if nc.partition_id_tensor else None)
        if self.partition_name and self.partition_name in in_names:
            in_names.remove(self.partition_name)
        self.in_names = in_names
        self.out_names = out_names
        self.out_avals = out_avals
        self.zero_outs = zero_outs
        n_params = len(in_names)
        self.n_params = n_params
        all_names = in_names + out_names
        if self.partition_name:
            all_names = all_names + [self.partition_name]

        def _body(*args):
            operands = list(args)
            if self.partition_name:
                operands.append(bass2jax.partition_id_tensor())
            return tuple(bass2jax._bass_exec_p.bind(
                *operands, out_avals=tuple(out_avals),
                in_names=tuple(all_names), out_names=tuple(out_names),
                lowering_input_output_aliases=(),
                sim_require_finite=True, sim_require_nnan=True, nc=nc))

        devices = jax.devices()[:n_cores]
        mesh = Mesh(np.asarray(devices), ("core",))
        self.mesh = mesh
        n_out = len(out_names)
        self.jitted = jax.jit(
            shard_map(_body, mesh=mesh,
                      in_specs=(PartitionSpec("core"),) * (n_params + n_out),
                      out_specs=(PartitionSpec("core"),) * n_out,
                      check_rep=False),
            keep_unused=True)
        self._jax = jax

    def prepare(self, in_maps):
        per_core = [[np.asarray(m[n]) for n in self.in_names]
                    for m in in_maps]
        concat_in = [
            np.concatenate([per_core[c][i] for c in range(self.n_cores)], 0)
            for i in range(self.n_params)]
        concat_zeros = [
            np.zeros((self.n_cores * z.shape[0], *z.shape[1:]), z.dtype)
            for z in self.zero_outs]
        # Pre-place on device with the mesh sharding so steady-state run()
        # measures device execution, not host->device tunnel transfers.
        from jax.sharding import NamedSharding, PartitionSpec
        sh = NamedSharding(self.mesh, PartitionSpec("core"))
        args = [self._jax.device_put(a, sh) for a in concat_in + concat_zeros]
        self._jax.block_until_ready(args)
        return args

    def run(self, args):
        outs = self.jitted(*args)
        self._jax.block_until_ready(outs)
        return outs

    def results(self, outs):
        return [
            {name: np.asarray(outs[i]).reshape(
                self.n_cores, *self.out_avals[i].shape)[c]
             for i, name in enumerate(self.out_names)}
            for c in range(self.n_cores)]


_RUNNER = None
_ARGS = None

# ---------------------------------------------------------------------------
# Entry point
# ---------------------------------------------------------------------------

def kernel(emb, W1, a1, W2, a2, edges):
    global _LAST_RESULTS, _RUNNER, _ARGS

    emb = np.asarray(emb)
    n_nodes = emb.shape[0]
    npc, stripe, _ = _cfg(n_nodes)

    arrays, c_max = _prep(np.asarray(edges), n_nodes)
    nc = _build_program(n_nodes, c_max)

    in_maps = []
    for c in range(NCORES):
        a = arrays[c]
        embT = np.zeros((D, stripe), ml_dtypes.bfloat16)
        embT[:, :npc] = emb[c * npc:(c + 1) * npc].T.astype(ml_dtypes.bfloat16)
        in_maps.append({
            "embT": embT,
            "iotaf": _IOTA_F,
            "identbf": _IDENT_BF,
            "W1": np.asarray(W1).astype(ml_dtypes.bfloat16),
            "W2": np.asarray(W2).astype(ml_dtypes.bfloat16),
            "Wa1": np.stack([np.asarray(W1) @ np.asarray(a1)[:D],
                             np.asarray(W1) @ np.asarray(a1)[D:]],
                            1).astype(ml_dtypes.bfloat16),
            "Wa2": np.stack([np.asarray(W2) @ np.asarray(a2)[:D],
                             np.asarray(W2) @ np.asarray(a2)[D:]],
                            1).astype(ml_dtypes.bfloat16),
            "xidx": a["xidx"], "obase": a["obase"], "oidx": a["oidx"],
            "scol": a["scol"],
        })

    runner = _Runner(nc, NCORES)
    args = runner.prepare(in_maps)
    results = runner.results(runner.run(args))
    _RUNNER, _ARGS = runner, args
    out = np.concatenate(
        [results[c]["out_stripe"][:npc] for c in range(NCORES)], 0)
    return out.astype(np.float32)



# revision 59
# speedup vs baseline: 1.1817x; 1.1817x over previous
"""Trainium2 Bass kernel for a 2-layer sparse GAT (nn_GAT_71889162600962).

Strategy (8 NeuronCores, SPMD):
- Nodes striped across cores (12500/core). Edges sharded by the core that
  owns their *src* node, so each core exclusively owns the segment sums
  (num/denom) of its stripe -- no all-reduce.
- Per layer, each core computes its stripe of h2 = h @ W (and the two
  attention projections s_src/s_dst = h2 . a halves) with bf16 PE matmuls,
  transposes h2 back to row-major "records"
      rec[n] = [h2[n] (256 bf16) | s_dst[n] | 1.0 | pad] (768B rows)
  plus a [stripe,1] f32 s_src side table, and all-gathers the record
  table across cores (vfull = 8*stripe rows).
- Edge phase: edges sorted by src, packed into chunks of <=128
  consecutive src nodes x 4096 edge slots, the slots split into 4
  1024-slot cells by dst quartile (dma_gather int16 indices; 1024 is the
  per-instruction descriptor-carveout limit). Per chunk: 4 dma_gather
  instructions fetch rec[dst] for all slots (slot i -> partition i%128,
  tile i//128); one indirect DMA with equal per-partition offsets
  broadcasts the chunk's 128-row s_src window to all partitions. Scores
  for all 32 tiles are then built as full [128e x 128s] matrices in 4
  fused DVE passes + 1 ACT exp:
      m[e,t,s] = (iota[s]==src_local[e,t]) * exp(-lrelu(ssrc[s]+sdst[e,t]))
  (stride-0 broadcast access patterns), and 32 PE matmuls accumulate
  num/denom in PSUM via psum[s,:] += m[:,t,:]^T @ X[:,t,:258]. Finalize
  (num/denom, ELU) per chunk; rows land via a 128-row indirect scatter.
  Pad slots point at the stripe-1 sentinel record whose s_dst is BIG, so
  exp(-lrelu(BIG+x)) == 0 kills their contribution.
- All per-core variation lives in input index arrays, so one SPMD program
  serves all 8 cores. Inputs are device_put once; steady-state reruns
  measure device execution (amortized over async dispatches).
"""

import math

import numpy as np
import ml_dtypes

P = 128
D = 256
REC_W = 384              # record row (bf16): 256 h2 | s_dst | 1.0 | s_src | pad
NCORES = 8
NEG_SLOPE = 0.2
BIG = 200.0              # sentinel score -> exp(-BIG) == 0 in f32
CELL = 1024              # slots per dst-quartile cell (dma_gather int16 idx)
NCELLS = 4
CHUNK_SLOTS = CELL * NCELLS   # 4096 edge slots per chunk (32 tiles of 128)
TILES_PER_CHUNK = CHUNK_SLOTS // P
TILES_PER_CELL = CELL // P

_LAST_RESULTS = None
TRACE = False

_IOTA_F = np.tile(np.arange(P, dtype=np.float32)[None, :], (P, 1))
_IDENT_BF = np.eye(P, dtype=np.float32).astype(ml_dtypes.bfloat16)


def _cfg(n_nodes):
    npc = n_nodes // NCORES
    stripe = math.ceil((npc + 44) / 128) * 128
    return npc, stripe, stripe * NCORES


# ---------------------------------------------------------------------------
# Host-side preprocessing
# ---------------------------------------------------------------------------

def _pack_core(src_l, dst_g, npc, stripe, qrows):
    """Pack one core's (src-local, dst-global-row) edges, sorted by src,
    into chunks of <=128 consecutive nodes whose per-dst-quartile edge
    counts each fit a CELL."""
    order = np.argsort(src_l, kind="stable")
    s = src_l[order]
    d = dst_g[order]
    q = d // qrows
    ne = len(s)

    starts = np.flatnonzero(np.r_[True, s[1:] != s[:-1]])
    seg_node = s[starts]
    seg_len = np.diff(np.r_[starts, ne])
    nseg = len(starts)
    seg_id = np.repeat(np.arange(nseg), seg_len)
    cnt = np.bincount(seg_id * NCELLS + q,
                      minlength=nseg * NCELLS).reshape(nseg, NCELLS)

    chunks = []  # (node_base, n_nodes, seg_start, seg_end)
    i = 0
    while i < nseg:
        base = int(seg_node[i])
        i0 = i
        cell = np.zeros(NCELLS, np.int64)
        while i < nseg and int(seg_node[i]) - base < P:
            nxt = cell + cnt[i]
            if (nxt > CELL).any():
                break
            cell = nxt
            i += 1
        chunks.append((base, int(seg_node[i - 1]) - base + 1, i0, i))
    return {"chunks": chunks, "s": s, "d": d, "q": q, "starts": starts,
            "seg_len": seg_len}


def _fill_arrays(packed, n_chunks, stripe, qrows):
    s, dd, qq = packed["s"], packed["d"], packed["q"]
    starts, seg_len = packed["starts"], packed["seg_len"]
    chunks = packed["chunks"]
    C = n_chunks
    sent = stripe - 1
    TPC = TILES_PER_CHUNK

    xidx = np.full((C, NCELLS, CELL), sent, np.int16)  # rec row rel. quartile
    scol = np.full((C, CHUNK_SLOTS), -1.0, np.float32)  # src - off in [0,128)
    oidx = np.full((C, P), sent, np.int32)
    obase = np.zeros((C, P), np.int32)                 # ssrc window base

    for k, (base, nn, i0, i1) in enumerate(chunks):
        e0 = int(starts[i0])
        e1 = int(starts[i1 - 1] + seg_len[i1 - 1])
        dce, sce, qce = dd[e0:e1], s[e0:e1], qq[e0:e1]
        off = min(base, stripe - P)
        for q in range(NCELLS):
            sel = qce == q
            n = int(sel.sum())
            xidx[k, q, :n] = (dce[sel] - q * qrows).astype(np.int16)
            scol[k, q * CELL:q * CELL + n] = (sce[sel] - off)
        lo = base - off
        oidx[k, lo:lo + nn] = base + np.arange(nn, dtype=np.int32)
        obase[k, :] = off

    # wrap16: index flat position i -> [i % 16, i // 16], replicated x8
    def w16(a):
        n = a.shape[-1]
        out = a.reshape(-1, n // 16, 16).transpose(0, 2, 1)
        return np.ascontiguousarray(
            np.tile(out, (1, 8, 1))).reshape(*a.shape[:-1], P, n // 16)

    # slot i -> (partition i % 128, tile i // 128): [C, 128, 32]
    def pb(a):
        return np.ascontiguousarray(
            a.reshape(C, TPC, P).transpose(0, 2, 1))

    return {"xidx": np.ascontiguousarray(w16(xidx).transpose(0, 2, 1, 3)),
            "scol": pb(scol),
            "oidx": np.ascontiguousarray(oidx[:, :, None]),
            "obase": np.ascontiguousarray(obase[:, :, None])}


def _prep(edges, n_nodes):
    npc, stripe, vfull = _cfg(n_nodes)
    qrows = vfull // NCELLS
    src = np.asarray(edges[0])
    dst = np.asarray(edges[1])
    core = src // npc
    dst_g = (dst // npc) * stripe + dst % npc

    packed = []
    for c in range(NCORES):
        sel = core == c
        packed.append(_pack_core(src[sel] - c * npc, dst_g[sel],
                                 npc, stripe, qrows))
    c_max = max(len(p["chunks"]) for p in packed)
    return [_fill_arrays(p, c_max, stripe, qrows) for p in packed], c_max


# ---------------------------------------------------------------------------
# Device program
# ---------------------------------------------------------------------------

def _build_program(n_nodes, n_chunks, mode=4, c_run=None):
    import concourse.bacc as bacc
    import concourse.mybir as mybir
    import concourse.tile as tile

    f32 = mybir.dt.float32
    bf16 = mybir.dt.bfloat16
    i16 = mybir.dt.int16
    Alu = mybir.AluOpType
    Act = mybir.ActivationFunctionType

    npc, stripe, vfull = _cfg(n_nodes)
    C = n_chunks
    NT = 512
    TPC = TILES_PER_CHUNK
    groups = [list(range(NCORES))]
    Off = __import__("concourse.bass", fromlist=["IndirectOffsetOnAxis"]).IndirectOffsetOnAxis

    nc = bacc.Bacc("TRN2", target_bir_lowering=False, debug=False,
                   num_devices=NCORES)

    embT_d = nc.dram_tensor("embT", [D, stripe], bf16, kind="ExternalInput")
    iota_d = nc.dram_tensor("iotaf", [P, P], f32, kind="ExternalInput")
    ident_d = nc.dram_tensor("identbf", [P, P], bf16, kind="ExternalInput")
    W_d = [nc.dram_tensor(f"W{L + 1}", [D, D], bf16, kind="ExternalInput")
           for L in range(2)]
    Wa_d = [nc.dram_tensor(f"Wa{L + 1}", [D, 2], bf16, kind="ExternalInput")
            for L in range(2)]
    i32 = mybir.dt.int32
    qrows = vfull // NCELLS
    xidx_d = nc.dram_tensor("xidx", [C, P, NCELLS, CELL // 16], i16,
                            kind="ExternalInput")
    oidx_d = nc.dram_tensor("oidx", [C, P, 1], i32, kind="ExternalInput")
    obase_d = nc.dram_tensor("obase", [C, P, 1], i32, kind="ExternalInput")
    scol_d = nc.dram_tensor("scol", [C, P, TPC], f32, kind="ExternalInput")
    out_d = nc.dram_tensor("out_stripe", [stripe, D], f32,
                           kind="ExternalOutput")

    rec_stripe = [nc.dram_tensor(f"rec_stripe{L}", [stripe, REC_W], bf16)
                  for L in range(2)]
    rec_full = [nc.dram_tensor(f"rec_full{L}", [vfull, REC_W], bf16,
                               addr_space="Shared") for L in range(2)]
    ssrc_loc = [nc.dram_tensor(f"ssrc_loc{L}", [stripe, 1], f32)
                for L in range(2)]
    out1rec = nc.dram_tensor("out1rec", [stripe, D], bf16)

    BAP = __import__("concourse.bass", fromlist=["AP"]).AP

    with tile.TileContext(nc) as tc:
        r_cell = nc.gpsimd.to_reg(CELL)
        with tc.tile_pool(name="const", bufs=1) as cpool:
            iota_f = cpool.tile([P, P], f32)
            nc.sync.dma_start(iota_f[:], iota_d[:])
            ident = cpool.tile([P, P], bf16)
            nc.sync.dma_start(ident[:], ident_d[:])
            W_sb, Wa_sb = [], []
            for L in range(2):
                w = cpool.tile([P, 2, D], bf16)
                wa = cpool.tile([P, 2, 2], bf16)
                for kc in range(2):
                    nc.sync.dma_start(w[:, kc, :], W_d[L][P * kc:P * (kc + 1)])
                    nc.sync.dma_start(wa[:, kc, :],
                                      Wa_d[L][P * kc:P * (kc + 1)])
                W_sb.append(w)
                Wa_sb.append(wa)

            for L in range(2 if mode >= 1 else 0):
                # ---------------- phase A: stripe matmul ------------------
                with (
                    tc.tile_pool(name=f"A{L}", bufs=3) as ap,
                    tc.tile_pool(name=f"As{L}", bufs=1) as spl,
                    tc.tile_pool(name=f"Ap{L}", bufs=2, space="PSUM") as pp,
                    tc.tile_pool(name=f"ApT{L}", bufs=4, space="PSUM") as ppT,
                ):
                    s_sbuf = spl.tile([3, stripe], f32)
                    nc.vector.memset(s_sbuf[:], 1.0)
                    s_bf = spl.tile([2, stripe], bf16)
                    ones_bf = spl.tile([1, stripe], bf16)
                    nc.vector.memset(ones_bf[:], 1.0)
                    for c0 in range(0, stripe, NT):
                        nsz = min(NT, stripe - c0)
                        hT = []
                        for kc in range(2):
                            t = ap.tile([P, nsz], bf16, tag=f"hT{kc}")
                            if L == 0:
                                nc.sync.dma_start(
                                    t[:], embT_d[P * kc:P * (kc + 1),
                                                 c0:c0 + nsz])
                            else:
                                nc.sync.dma_start_transpose(
                                    t[:], out1rec[c0:c0 + nsz,
                                                  P * kc:P * (kc + 1)])
                            hT.append(t)
                        ps_s = pp.tile([2, NT], f32, space="PSUM", tag="ps_s")
                        for kc in range(2):
                            nc.tensor.matmul(ps_s[:, :nsz],
                                             lhsT=Wa_sb[L][:, kc, :],
                                             rhs=hT[kc][:], start=kc == 0,
                                             stop=kc == 1)
                        nc.vector.tensor_copy(s_sbuf[0:2, c0:c0 + nsz],
                                              ps_s[:, :nsz])
                        nc.vector.tensor_copy(s_bf[0:2, c0:c0 + nsz],
                                              ps_s[0:2, :nsz])
                        rows = [ap.tile([P, D], bf16, tag=f"rows{b}",
                                        name=f"rows{b}")
                                for b in range(nsz // P)]
                        for j in range(2):
                            ps_h = pp.tile([P, NT], f32, space="PSUM",
                                           tag="ps_h")
                            for kc in range(2):
                                nc.tensor.matmul(
                                    ps_h[:, :nsz],
                                    lhsT=W_sb[L][:, kc, P * j:P * (j + 1)],
                                    rhs=hT[kc][:], start=kc == 0, stop=kc == 1)
                            h2T = ap.tile([P, nsz], bf16, tag=f"h2T{j}")
                            nc.vector.tensor_copy(h2T[:], ps_h[:, :nsz])
                            for b in range(nsz // P):
                                psT = ppT.tile([P, P], bf16, space="PSUM",
                                               tag="psT")
                                nc.tensor.transpose(
                                    out=psT[:], in_=h2T[:, P * b:P * (b + 1)],
                                    identity=ident[:])
                                nc.vector.tensor_copy(
                                    rows[b][:, P * j:P * (j + 1)], psT[:])
                        for b in range(nsz // P):
                            nc.scalar.dma_start(
                                rec_stripe[L][c0 + P * b:c0 + P * (b + 1),
                                              :D],
                                rows[b][:])
                    nc.vector.memset(s_sbuf[0:2, stripe - 1:stripe], BIG)
                    nc.vector.memset(s_bf[0:2, stripe - 1:stripe], BIG)
                    # s_src -> local table; s_dst, 1.0 -> record cols 256/257
                    # (bf16-staged so the DMAs stay on HWDGE engines)
                    nc.sync.dma_start(ssrc_loc[L][:, 0:1], s_sbuf[0:1, :])
                    nc.scalar.dma_start(rec_stripe[L][:, D:D + 1], s_bf[1:2, :])
                    nc.scalar.dma_start(rec_stripe[L][:, D + 1:D + 2],
                                        ones_bf[0:1, :])
                    if mode >= 2:
                        nc.gpsimd.collective_compute(
                            "AllGather", Alu.bypass, replica_groups=groups,
                            ins=[rec_stripe[L][:]], outs=[rec_full[L][:]])

                # ---------------- phase B: edge phase ---------------------
                if mode < 3:
                    continue
                tgt = out1rec if L == 0 else out_d
                stage_dt = bf16 if L == 0 else f32
                with (
                    tc.tile_pool(name=f"B{L}", bufs=3) as ep,
                    tc.tile_pool(name=f"Bx{L}", bufs=3) as xp,
                    tc.tile_pool(name=f"Bm{L}", bufs=2) as mp,
                    tc.tile_pool(name=f"Bo{L}", bufs=3) as ohp,
                    tc.tile_pool(name=f"Bf{L}", bufs=2) as fp,
                    tc.tile_pool(name=f"Bp{L}", bufs=2, space="PSUM") as pnp,
                ):
                    def front(ch):
                        """Loads + Pool gathers for one chunk (issued 2
                        chunks ahead so the scatter of chunk k never stalls
                        the Pool queue behind k's compute chain)."""
                        xi = ep.tile([P, NCELLS, CELL // 16], i16, tag="xi")
                        nc.sync.dma_start(xi[:], xidx_d[ch])
                        scl = ep.tile([P, TPC], f32, tag="scl")
                        nc.sync.dma_start(scl[:], scol_d[ch])
                        oi = ep.tile([P, 1], i32, tag="oi")
                        nc.sync.dma_start(oi[:], oidx_d[ch])
                        obs = ep.tile([P, 1], i32, tag="obs")
                        nc.sync.dma_start(obs[:], obase_d[ch])

                        # one-hot built up front: depends only on scl, so
                        # DVE does it while the gathers are in flight
                        scl_ap = scl[:, :]
                        scl_b = BAP(scl_ap.tensor, scl_ap.offset,
                                    [scl_ap.ap[0], scl_ap.ap[1], [0, P]])
                        io_ap = iota_f[:, :]
                        io_b = BAP(io_ap.tensor, io_ap.offset,
                                   [io_ap.ap[0], [0, TPC], io_ap.ap[1]])
                        m_all = ohp.tile([P, TPC, P], bf16, tag="m_all")
                        nc.vector.tensor_tensor(
                            out=m_all[:], in0=io_b, in1=scl_b,
                            op=Alu.is_equal)

                        # 768B record gather by dst (4 quartile cells) and
                        # the chunk's 128-row s_src window broadcast to all
                        # partitions (all indices equal -> 512B rows)
                        X = xp.tile([P, TPC, REC_W], bf16, tag="X")
                        ssb = ep.tile([P, P], f32, tag="ssb")
                        if mode >= 4:
                            TQ = TILES_PER_CELL
                            for cq in range(NCELLS):
                                nc.gpsimd.dma_gather(
                                    out_ap=X[:, TQ * cq:TQ * (cq + 1), :],
                                    in_ap=rec_full[L][qrows * cq:
                                                      qrows * (cq + 1), :],
                                    idxs_ap=xi[:, cq, :], num_idxs=CELL,
                                    num_idxs_reg=r_cell, elem_size=REC_W)
                            nc.gpsimd.indirect_dma_start(
                                out=ssb[:], out_offset=None,
                                in_=ssrc_loc[L][:],
                                in_offset=Off(ap=obs[:, 0:1], axis=0))
                        else:
                            nc.vector.memset(X[:], 0.0)
                            nc.vector.memset(ssb[:], BIG)
                        return X, ssb, m_all, oi

                    def back(X, ssb, m_all, oi):
                        # score matrices for all 32 tiles in 4 fused ops:
                        # sm[e,t,s] = lrelu(ssrc[off+s] + s_dst[dst_e])
                        # m[e,t,s] = (iota[s]==scl[e,t]) * exp(-sm)
                        sd_ap = X[:, :, D:D + 1]
                        sd_b = BAP(sd_ap.tensor, sd_ap.offset,
                                   [sd_ap.ap[0], sd_ap.ap[1], [0, P]])
                        sb_ap = ssb[:, :]
                        sb_b = BAP(sb_ap.tensor, sb_ap.offset,
                                   [sb_ap.ap[0], [0, TPC], sb_ap.ap[1]])
                        sm = mp.tile([P, TPC, P], bf16, tag="sm")
                        nc.vector.tensor_tensor(
                            out=sm[:], in0=sb_b, in1=sd_b, op=Alu.add)
                        nc.vector.scalar_tensor_tensor(
                            out=sm[:], in0=sm[:], scalar=NEG_SLOPE,
                            in1=sm[:], op0=Alu.mult, op1=Alu.max)
                        e2 = mp.tile([P, TPC, P], bf16, tag="e2")
                        nc.scalar.activation(e2[:], sm[:], Act.Exp,
                                             scale=-1.0)
                        nc.vector.tensor_tensor(
                            out=m_all[:], in0=m_all[:], in1=e2[:],
                            op=Alu.mult)

                        psum = pnp.tile([P, D + 2], f32, space="PSUM",
                                        tag="psum")
                        for t in range(TPC):
                            nc.tensor.matmul(
                                psum[:], lhsT=m_all[:, t, :],
                                rhs=X[:, t, :D + 2],
                                start=t == 0, stop=t == TPC - 1)
                        recip = fp.tile([P, 1], f32, tag="recip")
                        nc.vector.reciprocal(recip[:], psum[:, D + 1:D + 2])
                        q = fp.tile([P, D], f32, tag="q")
                        nc.vector.tensor_scalar_mul(q[:], psum[:, :D],
                                                    recip[:, :1])
                        amin = fp.tile([P, D], f32, tag="amin")
                        nc.vector.tensor_scalar(
                            out=amin[:], in0=q[:], scalar1=0.0,
                            scalar2=None, op0=Alu.min)
                        ea = fp.tile([P, D], f32, tag="ea")
                        nc.scalar.activation(ea[:], amin[:], Act.Exp)
                        bmax = fp.tile([P, D], f32, tag="bmax")
                        nc.vector.tensor_scalar(
                            out=bmax[:], in0=q[:], scalar1=0.0,
                            scalar2=-1.0, op0=Alu.max, op1=Alu.add)
                        stage = fp.tile([P, D], stage_dt, tag="stage")
                        nc.vector.tensor_tensor(
                            out=stage[:], in0=ea[:], in1=bmax[:],
                            op=Alu.add)
                        nc.gpsimd.indirect_dma_start(
                            out=tgt[:],
                            out_offset=Off(ap=oi[:, 0:1], axis=0),
                            in_=stage[:], in_offset=None)

                    CR = C if c_run is None else min(C, c_run)
                    pend = []
                    for ch in range(CR):
                        pend.append(front(ch))
                        if len(pend) > 2:
                            back(*pend.pop(0))
                    for item in pend:
                        back(*item)
    nc.compile()
    return nc



# ---------------------------------------------------------------------------
# Persistent-jit PJRT runner (NTFF profiling is unavailable under this axon
# setup, so steady-state re-execution wall clock is the timing source).
# ---------------------------------------------------------------------------

class _Runner:
    def __init__(self, nc, n_cores):
        import jax
        from jax.sharding import Mesh, PartitionSpec
        from jax.experimental.shard_map import shard_map
        import concourse.mybir as mybir
        from concourse import bass2jax

        bass2jax.install_neuronx_cc_hook()
        self.n_cores = n_cores
        in_names, out_names, out_avals, zero_outs = [], [], [], []
        for alloc in nc.m.functions[0].allocations:
            if not isinstance(alloc, mybir.MemoryLocationSet):
                continue
            name = alloc.memorylocations[0].name
            if alloc.kind == "ExternalInput":
                in_names.append(name)
            elif alloc.kind == "ExternalOutput":
                out_names.append(name)
                shape = tuple(alloc.tensor_shape)
                dtype = mybir.dt.np(alloc.dtype)
                out_avals.append(jax.core.ShapedArray(shape, dtype))
                zero_outs.append(np.zeros(shape, dtype))
        self.partition_name = (nc.partition_id_tensor.name
                               if nc.partition_id_tensor else None)
        if self.partition_name and self.partition_name in in_names:
            in_names.remove(self.partition_name)
        self.in_names = in_names
        self.out_names = out_names
        self.out_avals = out_avals
        self.zero_outs = zero_outs
        n_params = len(in_names)
        self.n_params = n_params
        all_names = in_names + out_names
        if self.partition_name:
            all_names = all_names + [self.partition_name]

        def _body(*args):
            operands = list(args)
            if self.partition_name:
                operands.append(bass2jax.partition_id_tensor())
            return tuple(bass2jax._bass_exec_p.bind(
                *operands, out_avals=tuple(out_avals),
                in_names=tuple(all_names), out_names=tuple(out_names),
                lowering_input_output_aliases=(),
                sim_require_finite=True, sim_require_nnan=True, nc=nc))

        devices = jax.devices()[:n_cores]
        mesh = Mesh(np.asarray(devices), ("core",))
        self.mesh = mesh
        n_out = len(out_names)
        self.jitted = jax.jit(
            shard_map(_body, mesh=mesh,
                      in_specs=(PartitionSpec("core"),) * (n_params + n_out),
                      out_specs=(PartitionSpec("core"),) * n_out,
                      check_rep=False),
            keep_unused=True)
        self._jax = jax

    def prepare(self, in_maps):
        per_core = [[np.asarray(m[n]) for n in self.in_names]
                    for m in in_maps]
        concat_in = [
            np.concatenate([per_core[c][i] for c in range(self.n_cores)], 0)
            for i in range(self.n_params)]
        concat_zeros = [
            np.zeros((self.n_cores * z.shape[0], *z.shape[1:]), z.dtype)
            for z in self.zero_outs]
        # Pre-place on device with the mesh sharding so steady-state run()
        # measures device execution, not host->device tunnel transfers.
        from jax.sharding import NamedSharding, PartitionSpec
        sh = NamedSharding(self.mesh, PartitionSpec("core"))
        args = [self._jax.device_put(a, sh) for a in concat_in + concat_zeros]
        self._jax.block_until_ready(args)
        return args

    def run(self, args):
        outs = self.jitted(*args)
        self._jax.block_until_ready(outs)
        return outs

    def results(self, outs):
        return [
            {name: np.asarray(outs[i]).reshape(
                self.n_cores, *self.out_avals[i].shape)[c]
             for i, name in enumerate(self.out_names)}
            for c in range(self.n_cores)]


_RUNNER = None
_ARGS = None

# ---------------------------------------------------------------------------
# Entry point
# ---------------------------------------------------------------------------

def kernel(emb, W1, a1, W2, a2, edges):
    global _LAST_RESULTS, _RUNNER, _ARGS

    emb = np.asarray(emb)
    n_nodes = emb.shape[0]
    npc, stripe, _ = _cfg(n_nodes)

    arrays, c_max = _prep(np.asarray(edges), n_nodes)
    nc = _build_program(n_nodes, c_max)

    in_maps = []
    for c in range(NCORES):
        a = arrays[c]
        embT = np.zeros((D, stripe), ml_dtypes.bfloat16)
        embT[:, :npc] = emb[c * npc:(c + 1) * npc].T.astype(ml_dtypes.bfloat16)
        in_maps.append({
            "embT": embT,
            "iotaf": _IOTA_F,
            "identbf": _IDENT_BF,
            "W1": np.asarray(W1).astype(ml_dtypes.bfloat16),
            "W2": np.asarray(W2).astype(ml_dtypes.bfloat16),
            "Wa1": np.stack([np.asarray(W1) @ np.asarray(a1)[:D],
                             np.asarray(W1) @ np.asarray(a1)[D:]],
                            1).astype(ml_dtypes.bfloat16),
            "Wa2": np.stack([np.asarray(W2) @ np.asarray(a2)[:D],
                             np.asarray(W2) @ np.asarray(a2)[D:]],
                            1).astype(ml_dtypes.bfloat16),
            "xidx": a["xidx"], "obase": a["obase"], "oidx": a["oidx"],
            "scol": a["scol"],
        })

    runner = _Runner(nc, NCORES)
    args = runner.prepare(in_maps)
    results = runner.results(runner.run(args))
    _RUNNER, _ARGS = runner, args
    out = np.concatenate(
        [results[c]["out_stripe"][:npc] for c in range(NCORES)], 0)
    return out.astype(np.float32)

